# revision 26
# baseline (speedup 1.0000x reference)
"""Multi-head scaled-dot-product attention (ABSA-style, per-head projections)
on 8 Trainium2 NeuronCores.

Reference computation (per head h, batch b):
    kx = k @ w_kx[h]                    # (512, 96)
    qx = q @ w_qx[h]                    # (512, 96)
    s  = qx @ kx.T / sqrt(96)           # (512, 512)
    a  = softmax(s, axis=-1)
    o  = a @ kx                         # (512, 96)
    out[b, :, h*96:(h+1)*96] = o

Distribution: data-parallel over batch, 4 batches per core, all heads on
every core.  The host pre-transposes k/q to (embed, seq), pre-casts all
inputs to bf16, and concatenates the per-core outputs.

Per-core dataflow (matmuls in bf16, accumulation + softmax math in f32):
  - q-projections run per head with the (embed, hid) weight chunk
    stationary, producing qx^T (hid on partitions) for the score streams.
  - k-projections run PACKED across heads in natural layout: the kT
    seq-chunk is stationary (full 128x128) and the 8-head weight matrix
    streams 384 columns at a time, producing kx[seq, head*96] directly in
    the layout the attention-weighted sum needs.  This covers all heads in
    48 N=384 streams instead of 48 N=512 streams (25% fewer PE cycles),
    and the eviction drops straight into the kxo tiles (ones column
    appended for the softmax-denominator trick).
  - Score stationaries kx^T[96,128] are built by cheap PE transposes of
    the natural kx chunks (~60ns each in a warm stream).
  - Scores are computed transposed, s^T (k, q), so the softmax reduction
    axis lands on PSUM partitions and is folded into the attention matmul
    via the kxo ones-column.  exp() runs unshifted (scores are O(1)).
  - o^T = kxo^T @ exp, then 4 PE transposes per head put o back into
    (queries, hid) layout where normalisation runs batched (reciprocal +
    stride-0-broadcast multiply).  Output is written bf16.
  - Startup: warm-up matmuls on the identity hold the PE's HAM activity
    window (transpose-mode work does not count) while q0^T and the
    q-weights land; all loads share the single gpsimd DMA queue in need-order (a second
    queue round-robins the DMA engines and starves the weights); a dummy
    exp preloads the ACT spline table off the critical path.
"""

import math
from functools import lru_cache

import numpy as np

import concourse.bass as bass
import concourse.tile as tile
from concourse import mybir
from concourse.bass_utils import run_bass_kernel_spmd

# ---------------------------------------------------------------------------
# Workaround for walrus "Too many sync wait commands": some instruction
# encodings accept only a single sync-wait, but Tile can attach several.
# Hoist every wait beyond the first onto a same-engine no-op inserted right
# before the instruction — program order on the engine makes that equivalent.
# ---------------------------------------------------------------------------

import bass_rust as _bass_rust


def _split_excess_waits(nc, max_waits=1):
    n = 0
    for f in nc.m.functions:
        for bb in f.blocks:
            il = bb.instructions
            i = 0
            while i < len(il):
                ins = il[i]
                si = ins.sync_info
                waits = list(si.on_wait or []) if si is not None else []
                if len(waits) > max_waits:
                    si.on_wait = waits[:max_waits]
                    for w in waits[max_waits:]:
                        nop = mybir.InstNoOp(name=f"waitnop-{n}", ins=[],
                                             outs=[])
                        n += 1
                        nop.engine = ins.engine
                        nop.sync_info = _bass_rust.SyncInfo(
                            on_wait=[w], on_update=[])
                        il.insert(i, nop)
                        i += 1
                i += 1

# ---------------------------------------------------------------------------
# Problem constants (full problem; hardcoded per the harness contract)
# ---------------------------------------------------------------------------
EMBED = 768
HID = 96
N_HEAD = 8
BATCH = 32
SEQ = 512
N_CORES = 8
B = BATCH // N_CORES  # batches per core
EC = EMBED // 128  # embed chunks of 128
KC = SEQ // 128  # key chunks of 128
QC = SEQ // 128  # query chunks of 128
HH = N_HEAD // 2 * HID  # 384: half of the packed head-hid dim
SCALE = 1.0 / math.sqrt(HID)

F32 = mybir.dt.float32
BF16 = mybir.dt.bfloat16


def build_bass():
    nc = bass.Bass("TRN2", target_bir_lowering=False, debug=False,
                   num_devices=N_CORES)

    # host ships k/q pre-transposed to (embed, seq) and pre-cast to bf16
    k_in = nc.declare_dram_parameter("k", [B, EMBED, SEQ], BF16,
                                     isOutput=False)
    q_in = nc.declare_dram_parameter("q", [B, EMBED, SEQ], BF16,
                                     isOutput=False)
    # w_kx packed for the natural-layout projection: (partition, ec, 768)
    # where the last axis is h*96+j (all heads side by side)
    wk_in = nc.declare_dram_parameter("w_kx", [128, EC, N_HEAD * HID], BF16,
                                      isOutput=False)
    # w_qx per-head layout (partition, h*EC+ec, hid) for the ^T projection
    wq_in = nc.declare_dram_parameter("w_qx", [128, EC * N_HEAD, HID], BF16,
                                      isOutput=False)
    out_d = nc.declare_dram_parameter("out", [B, SEQ, EMBED], BF16,
                                      isOutput=True)
    id_in = nc.declare_dram_parameter("ident", [128, 128], BF16,
                                      isOutput=False)

    with nc.allow_low_precision("bf16 compute, f32 accumulate"), \
            tile.TileContext(nc) as tc:
        with tc.tile_pool(name="singles", bufs=1) as singles, \
                tc.tile_pool(name="kqt", bufs=1) as kqt_pool, \
                tc.tile_pool(name="wsb", bufs=1) as w_pool, \
                tc.tile_pool(name="stage", bufs=1) as stage_pool, \
                tc.tile_pool(name="exp", bufs=16) as exp_pool, \
                tc.tile_pool(name="kxo", bufs=8) as kxo_pool, \
                tc.tile_pool(name="kxt", bufs=10) as kxt_pool, \
                tc.tile_pool(name="ot", bufs=3) as ot_pool, \
                tc.tile_pool(name="recip", bufs=4) as recip_pool, \
                tc.tile_pool(name="ps_proj", bufs=2, space="PSUM") as ps_proj, \
                tc.tile_pool(name="ps_score", bufs=2, space="PSUM") as ps_score, \
                tc.tile_pool(name="ps_trans", bufs=2, space="PSUM") as ps_trans:

            # persistent per-(parity, head) qx^T tiles
            qxs = [[singles.tile([HID, SEQ], BF16, tag=f"qxs_{i}_{h}",
                                 name=f"qxs_{i}_{h}")
                    for h in range(N_HEAD)] for i in range(2)]

            # --- input pipeline -------------------------------------------
            WHB = EC * N_HEAD // 2
            wq_sb = [w_pool.tile([128, WHB, HID], BF16,
                                 tag=f"wq_{half}", name=f"wq_{half}")
                     for half in range(2)]
            wk_sb = [w_pool.tile([128, EC, HH], BF16,
                                 tag=f"wk_{half}", name=f"wk_{half}")
                     for half in range(2)]
            kqt = {}

            def wq_chunk(h, ec):
                blk = h * EC + ec
                return wq_sb[blk // WHB][:, blk % WHB, :]

            def load_kq(b, t, split=False):
                src_d = (k_in, q_in)[t]
                tt = kqt_pool.tile([128, EC, SEQ], BF16,
                                   tag=f"kq{t}_{b}", name=f"kq{t}_{b}")
                src = src_d[b].rearrange("(ec p) s -> p ec s", p=128)
                if split:
                    nc.gpsimd.dma_start(out=tt[:, 0:EC // 2, :],
                                        in_=src[:, 0:EC // 2, :])
                    nc.gpsimd.dma_start(out=tt[:, EC // 2:, :],
                                        in_=src[:, EC // 2:, :])
                else:
                    nc.gpsimd.dma_start(out=tt[:], in_=src)
                kqt[(b, t)] = tt

            # ALL loads go on the single gpsimd queue, in need-order: the
            # DMA engines drain one queue roughly FIFO, so a single queue
            # guarantees arrival order matches consumption order.  (Weight
            # loads on a second queue measured ~3us PE stalls: the queues
            # round-robin the shared engines, so the big b1 input loads
            # starved the wk transfer the first k-projection needed.)
            identity = singles.tile([128, 128], BF16, tag="identity")
            nc.gpsimd.dma_start(out=identity[:], in_=id_in[:, :])
            nc.gpsimd.dma_start(out=wq_sb[0][:], in_=wq_in[:, 0:WHB, :])
            load_kq(0, 1)
            nc.gpsimd.dma_start(out=wq_sb[1][:], in_=wq_in[:, WHB:, :])
            load_kq(0, 0)
            nc.gpsimd.dma_start(out=wk_sb[0][:], in_=wk_in[:, :, 0:HH])
            nc.gpsimd.dma_start(out=wk_sb[1][:], in_=wk_in[:, :, HH:])
            load_kq(1, 1)
            load_kq(1, 0)

            # preload the ACT exp spline table off the critical path
            dummy_e = singles.tile([1, 16], BF16, tag="dummy_e")
            nc.scalar.activation(dummy_e[:], identity[0:1, 0:16],
                                 mybir.ActivationFunctionType.Exp,
                                 scale=SCALE)

            # PE warm-up: real matmuls (transpose-mode PE work does not
            # count as busy for the HAM clock gate) keep the PE's activity
            # window hot while the q0^T/weight DMAs land, so the first
            # projections run at 2.4 GHz instead of 1.2 GHz.
            warm_ps = ps_score.tile([128, 2, SEQ], F32, tag="score",
                                    name="warm_ps")
            for _ in range(64):
                nc.tensor.matmul(warm_ps[:, 0, 0:128], identity[:],
                                 identity[:], start=True, stop=True)

            # Output staging: one (128, QC, EMBED) bf16 tile per parity.
            stage = [stage_pool.tile([128, QC, EMBED], BF16, tag=f"st{p}",
                                     name=f"st{p}")
                     for p in range(2)]

            # --- main loop ------------------------------------------------
            for b in range(B):
                par = b % 2
                st = stage[par]
                last = b == B - 1

                if b + 2 < B:
                    load_kq(b + 2, 1)
                    load_kq(b + 2, 0)

                def split_evict(dst, src, n):
                    # two half-copies on different engines: halves the
                    # eviction latency so the 2-buffer proj-PSUM ring never
                    # stalls the PE.  In the last batch both halves go to
                    # the vector engine: the tail is paced by the scalar
                    # engine's exp stream.
                    nc.vector.tensor_copy(dst[:, 0:n // 2], src[:, 0:n // 2])
                    if last:
                        nc.vector.tensor_copy(dst[:, n // 2:], src[:, n // 2:])
                    else:
                        nc.scalar.copy(dst[:, n // 2:], src[:, n // 2:])

                # q-projection phase (per head, qx^T for the score streams)
                for h in range(N_HEAD):
                    qx_ps = ps_proj.tile([HID, SEQ], F32, tag="proj",
                                         name="proj_ps")
                    for ec in range(EC):
                        nc.tensor.matmul(qx_ps[:], wq_chunk(h, ec),
                                         kqt[(b, 1)][:, ec, :],
                                         start=(ec == 0), stop=(ec == EC - 1))
                    split_evict(qxs[par][h], qx_ps, SEQ)

                # k-projection (packed natural layout) + scores
                kxo_all = {}
                kxT = {}
                exps = {}

                def emit_kproj(half, sc):
                    # kx[seq sc, heads 4*half..4*half+4] for all 4 heads of
                    # the half in one 6-matmul accumulation chain
                    kp_ps = ps_proj.tile([128, 4, HID], F32, tag="proj",
                                         name="kp_ps")
                    for ec in range(EC):
                        nc.tensor.matmul(
                            kp_ps[:],
                            kqt[(b, 0)][:, ec, sc * 128:(sc + 1) * 128],
                            wk_sb[half][:, ec, :],
                            start=(ec == 0), stop=(ec == EC - 1))
                    if sc not in kxo_all:
                        kxo = kxo_pool.tile([128, N_HEAD, HID + 2], BF16,
                                            tag="kxo", name="kxo")
                        nc.vector.memset(kxo[:, :, HID:HID + 1], 1.0)
                        kxo_all[sc] = kxo
                    kxo = kxo_all[sc]
                    # split eviction across engines (2 heads each)
                    nc.vector.tensor_copy(
                        kxo[:, 4 * half:4 * half + 2, 0:HID],
                        kp_ps[:, 0:2, :])
                    ev2 = (kxo[:, 4 * half + 2:4 * half + 4, 0:HID],
                           kp_ps[:, 2:4, :])
                    if last:
                        nc.vector.tensor_copy(*ev2)
                    else:
                        nc.scalar.copy(*ev2)

                def emit_kxt(h):
                    # score stationaries kx^T[96,128] via PE transposes of
                    # the natural kx chunks
                    tr_ps = ps_trans.tile([HID, KC, 128], BF16, tag="tr",
                                          name="tr_ps")
                    for kc in range(KC):
                        nc.tensor.transpose(
                            tr_ps[:, kc, :],
                            kxo_all[kc][:, h, 0:HID],
                            identity[:])
                    kt = kxt_pool.tile([HID, KC, 128], BF16, tag="kxt",
                                       name="kxt")
                    nc.vector.tensor_copy(kt[:], tr_ps[:])
                    kxT[h] = kt

                def emit_score(h):
                    qx_sb = qxs[par][h]
                    tiles = []
                    for shalf in range(2):
                        s_ps = ps_score.tile([128, 2, SEQ], F32, tag="score",
                                             name="s_ps")
                        for i in range(2):
                            kc = shalf * 2 + i
                            nc.tensor.matmul(
                                s_ps[:, i, :], kxT[h][:, kc, :],
                                qx_sb[:], start=True, stop=True)
                        e_sb = exp_pool.tile([128, 2, SEQ], BF16, tag="exp",
                                             name="e_sb")
                        nc.scalar.activation(
                            e_sb[:], s_ps[:],
                            mybir.ActivationFunctionType.Exp, scale=SCALE)
                        tiles.append(e_sb)
                    exps[h] = tiles

                # attention helpers (the last batch interleaves them with
                # the score phase, so define them up front)
                def wsum_head(h):
                    oT_ps = ps_proj.tile([HID + 1, SEQ], F32, tag="proj",
                                         name="oT_ps")
                    for kc in range(KC):
                        nc.tensor.matmul(
                            oT_ps[:], kxo_all[kc][:, h, 0:HID + 1],
                            exps[h][kc // 2][:, kc % 2, :],
                            start=(kc == 0), stop=(kc == KC - 1))
                    oT_sb = ot_pool.tile([HID + 1, SEQ], BF16, tag="ot",
                                         name="oT_sb")
                    # evict in halves: the norm transposes read one
                    # 128-col slice each, so they start as soon as the
                    # first half lands (~350ns earlier); odd heads in the
                    # last batch put the second half on the scalar engine
                    nc.vector.tensor_copy(oT_sb[:, 0:SEQ // 2],
                                          oT_ps[:, 0:SEQ // 2])
                    if last and h % 2:
                        nc.scalar.copy(oT_sb[:, SEQ // 2:],
                                       oT_ps[:, SEQ // 2:])
                    else:
                        nc.vector.tensor_copy(oT_sb[:, SEQ // 2:],
                                              oT_ps[:, SEQ // 2:])
                    return oT_sb

                def norm_head(h, oT_sb):
                    # HID+2 inner extent keeps the bf16 PSUM row stride
                    # 4-byte aligned; column 97 is dead padding
                    ob_ps = ps_trans.tile([128, QC, HID + 2], BF16, tag="tr",
                                          name="ob_ps")
                    rc = recip_pool.tile([128, QC, 1], F32, tag="recip",
                                         name="recip")
                    rc_ap = rc[:]
                    out_v = out_d[b].rearrange("(qc p) e -> p qc e", p=128)
                    # flush finished columns so stores overlap compute;
                    # the last batch flushes per head to shorten the tail
                    flush = (h % 2 == 1) if not last else True
                    lo = (h - 1 if h % 2 else h) * HID
                    # the terminal head's chain IS the kernel tail: run it
                    # per qc-pair so the first half normalises and stores
                    # while the second half is still evicting
                    pieces = ((0, QC),)
                    for plo, phi in pieces:
                        for qc in range(plo, phi):
                            nc.tensor.transpose(
                                ob_ps[:, qc, 0:HID + 1],
                                oT_sb[:, qc * 128:(qc + 1) * 128],
                                identity[0:HID + 1, 0:HID + 1])
                        nc.vector.reciprocal(rc[:, plo:phi],
                                             ob_ps[:, plo:phi, HID:HID + 1])
                        rc_b = bass.AP(rc_ap.tensor,
                                       rc_ap.offset + plo * rc_ap.ap[1][0],
                                       [rc_ap.ap[0],
                                        [rc_ap.ap[1][0], phi - plo],
                                        [0, HID]])
                        nc.vector.tensor_tensor(
                            st[:, plo:phi, h * HID:(h + 1) * HID],
                            ob_ps[:, plo:phi, 0:HID], rc_b,
                            mybir.AluOpType.mult)
                        if flush:
                            # last two stores go on the (idle) scalar DMA
                            # queue: the sync queue showed a ~2.5us issue
                            # stall on the final store
                            eng = nc.scalar if (last and h >= 6) else nc.sync
                            eng.dma_start(
                                out=out_v[:, plo:phi, lo:(h + 1) * HID],
                                in_=st[:, plo:phi, lo:(h + 1) * HID])

                oT = {}
                if not last:
                    # steady state: scores trail the kx pipeline; exps have
                    # a full attention phase of slack
                    for sc in range(KC):
                        emit_kproj(0, sc)
                    emit_kproj(1, 0)
                    emit_kxt(0)
                    emit_kproj(1, 1)
                    emit_kxt(1)
                    emit_score(0)
                    emit_kproj(1, 2)
                    emit_kxt(2)
                    emit_score(1)
                    emit_kproj(1, 3)
                    emit_kxt(3)
                    emit_score(2)
                    for h in range(4, N_HEAD):
                        emit_kxt(h)
                        emit_score(h - 1)
                    emit_score(N_HEAD - 1)
                    for h in range(N_HEAD):
                        oT[h] = wsum_head(h)
                        if h >= 1:
                            norm_head(h - 1, oT[h - 1])
                    norm_head(N_HEAD - 1, oT[N_HEAD - 1])
                else:
                    # tail: exps pace the score phase (the 2-deep score
                    # PSUM ring waits on the ~1.1us EXP activations), so
                    # start the early heads' weighted sums between the
                    # later heads' scores to keep the PE busy, and let the
                    # normalise chain trail by one head
                    for sc in range(KC):
                        emit_kproj(0, sc)
                    emit_kxt(0)
                    emit_kxt(1)
                    emit_score(0)
                    emit_kxt(2)
                    emit_score(1)
                    emit_kxt(3)
                    emit_score(2)
                    emit_kproj(1, 0)
                    emit_score(3)
                    emit_kproj(1, 1)
                    emit_kproj(1, 2)
                    emit_kproj(1, 3)
                    emit_kxt(4)
                    oT[0] = wsum_head(0)
                    emit_score(4)
                    emit_kxt(5)
                    oT[1] = wsum_head(1)
                    norm_head(0, oT[0])
                    emit_score(5)
                    emit_kxt(6)
                    oT[2] = wsum_head(2)
                    norm_head(1, oT[1])
                    emit_score(6)
                    emit_kxt(7)
                    oT[3] = wsum_head(3)
                    norm_head(2, oT[2])
                    emit_score(7)
                    for h in range(4, N_HEAD):
                        oT[h] = wsum_head(h)
                        norm_head(h - 1, oT[h - 1])
                    norm_head(N_HEAD - 1, oT[N_HEAD - 1])

    _split_excess_waits(nc)
    return nc


@lru_cache(maxsize=1)
def _get_nc():
    return build_bass()


def kernel(k, q, w_kx, w_qx):
    import ml_dtypes

    bf16 = ml_dtypes.bfloat16

    k = np.asarray(k, dtype=np.float32)
    q = np.asarray(q, dtype=np.float32)

    # pre-transpose k/q to (embed, seq) per batch and pre-cast to bf16 so
    # the device needs no on-chip input transposes and half the DMA bytes
    kT = np.ascontiguousarray(k.transpose(0, 2, 1).astype(bf16))
    qT = np.ascontiguousarray(q.transpose(0, 2, 1).astype(bf16))

    # w_qx: (partition, h*EC+ec, hid) per-head layout for the ^T projection
    w_qx = np.asarray(w_qx, dtype=np.float32)
    wq = np.ascontiguousarray(
        w_qx.reshape(8, 6, 128, 96).transpose(2, 0, 1, 3).reshape(
            128, 48, 96).astype(bf16))

    # w_kx: packed (partition, ec, h*96+j) for the natural-layout projection
    w_kx = np.asarray(w_kx, dtype=np.float32)
    # (h, e, j) -> (e, h*96+j): [768, 768], then split e into (ec, p)
    wall = w_kx.transpose(1, 0, 2).reshape(768, 768)
    wk = np.ascontiguousarray(
        wall.reshape(6, 128, 768).transpose(1, 0, 2).astype(bf16))

    ident = np.ascontiguousarray(np.eye(128, dtype=bf16))

    nc = _get_nc()
    in_maps = []
    for c in range(N_CORES):
        sl = slice(c * B, (c + 1) * B)
        in_maps.append({
            "k": np.ascontiguousarray(kT[sl]),
            "q": np.ascontiguousarray(qT[sl]),
            "w_kx": wk,
            "w_qx": wq,
            "ident": ident,
        })
    res = run_bass_kernel_spmd(nc, in_maps, core_ids=list(range(N_CORES)))
    return np.concatenate(
        [res.results[c]["out"].astype(np.float32) for c in range(N_CORES)],
        axis=0)
